# revision 7
# baseline (speedup 1.0000x reference)
"""Bass/Trainium2 kernel for a 16-head causal MHA block with partial rotary.

Problem shapes (hardcoded): x [2,2048,1024] fp32, Wq/Wk/Wv/Wo [1024,1024],
mask = causal tril [2048,2048] (hardcoded causality; mask input unused).

Sharding over 8 NeuronCores: core c handles batch c//4 and the 4 heads
h0 = (c%4)*4 .. h0+3 (tensor parallel on heads).  Each core computes its
partial output y_h @ Wo[h-block] summed over its 4 heads; the host adds the
4 per-batch partials.

Device-side plan (per core):
  A)  x [2048,1024] -> xT in SBUF via PE transposes (f32r transpose mode).
  B)  qT/kT = (Wslice^T @ xT) in [dims, seq] layout, v in [seq, dims] layout
      (+1.0 ones column per head for fused softmax denominators).
  B2) rotary applied to qT/kT in transposed layout:
      rot = q*C + pairswap(q)*S with host-precomputed C/S tables
      (identity rows for the non-rotary half of each head).
  C)  per (head, 512-wide i-chunk): logits^T tiles [j=128, i=512] = k^T q
      (f32r matmuls), exp via ScalarE (scale=1/8 folded in, no max
      subtraction -- logits are O(1)), causal zeroing of diagonal tiles via
      gpsimd affine_select, AV matmul with the ones column producing
      [65, 512] = [y^T ; colsums], then normalize by broadcasting 1/s.
  D)  output projection out = y^T.T @ Wo accumulated over the 4 local heads
      in PSUM, evicted and DMA'd to DRAM.
"""

import numpy as np

S, D, H, HD, PROT = 2048, 1024, 16, 64, 32
NHC = 4            # heads per core
SEQT = S // 128    # 16
DCH = D // 128     # 8
NIC = 4            # i-chunks of 512

_CACHED = {}


def _rot_tables():
    invf = 10000.0 ** (-np.arange(0, PROT, 2, dtype=np.float64) / PROT)  # [16]
    ang = np.arange(S, dtype=np.float64)[None, :] * invf[:, None]        # [16, S]
    C64 = np.ones((64, S), np.float64)
    S64 = np.zeros((64, S), np.float64)
    for d in range(PROT):
        C64[d] = np.cos(ang[d // 2])
        S64[d] = (1.0 if d % 2 else -1.0) * np.sin(ang[d // 2])
    CT = np.concatenate([C64, C64], 0).astype(np.float32)
    ST = np.concatenate([S64, S64], 0).astype(np.float32)
    return CT, ST


def build_nc():
    import concourse.bacc as bacc
    import concourse.mybir as mybir
    from concourse.tile import TileContext

    F32 = mybir.dt.float32
    F32R = mybir.dt.float32r
    AF = mybir.ActivationFunctionType
    ALU = mybir.AluOpType

    MM_MODE = "f32"  # "f32" = plain fp32 matmuls (4 cyc/row), "f32r" = reduced-precision full-rate

    def r(ap):  # matmul-operand dtype view
        if MM_MODE == "f32r":
            return ap.bitcast(F32R)
        return ap

    nc = bacc.Bacc("TRN2", target_bir_lowering=False, debug=False)

    x_d = nc.dram_tensor("x", [S, D], F32, kind="ExternalInput").ap()
    wq_d = nc.dram_tensor("wq", [D, 256], F32, kind="ExternalInput").ap()
    wk_d = nc.dram_tensor("wk", [D, 256], F32, kind="ExternalInput").ap()
    wv_d = nc.dram_tensor("wv", [D, 256], F32, kind="ExternalInput").ap()
    wo_d = nc.dram_tensor("wo", [256, D], F32, kind="ExternalInput").ap()
    out_d = nc.dram_tensor("out", [S, D], F32, kind="ExternalOutput").ap()

    CT, ST = _rot_tables()
    ct_d = nc.inline_tensor(CT, "ct_const").ap()
    st_d = nc.inline_tensor(ST, "st_const").ap()
    id_d = nc.inline_tensor(np.eye(128, dtype=np.float32), "id_const").ap()

    SWAP_MASK = [i ^ 1 for i in range(32)]

    with TileContext(nc) as tc:
        with (
            tc.tile_pool(name="persist", bufs=1) as pp,
            tc.tile_pool(name="small", bufs=2) as sp,
        ):
            qT = [pp.tile([128, S], F32, tag=f"qT{i}", name=f"qT{i}") for i in range(2)]
            kT = [pp.tile([128, S], F32, tag=f"kT{i}", name=f"kT{i}") for i in range(2)]
            vt = [pp.tile([128, NHC * 65], F32, tag=f"vt{i}", name=f"vt{i}") for i in range(SEQT)]
            wo_sb = [pp.tile([128, D], F32, tag=f"wo{i}", name=f"wo{i}") for i in range(2)]
            yT = [pp.tile([128, S], F32, tag=f"yT{i}", name=f"yT{i}") for i in range(2)]
            ident = pp.tile([128, 128], F32, tag="ident")
            nc.sync.dma_start(out=ident[:], in_=id_d[:])
            for i in range(2):
                nc.sync.dma_start(out=wo_sb[i][:], in_=wo_d[i * 128:(i + 1) * 128, :])

            # ---------------- Phase A+B: transpose x, projections ----------
            with (
                tc.tile_pool(name="xnat", bufs=2) as xp,
                tc.tile_pool(name="xT", bufs=1) as xTp,
                tc.tile_pool(name="wts", bufs=8) as wp,
            ):
                xT = xTp.tile([128, DCH * S], F32, tag="xT")  # chunk d at cols [d*S, (d+1)*S)
                xT3 = xT[:].rearrange("p (d s) -> p d s", d=DCH, s=S)

                with tc.tile_pool(name="psA", bufs=3, space="PSUM") as psA:
                    for st in range(SEQT):
                        xt = xp.tile([128, D], F32, tag="x")
                        nc.sync.dma_start(out=xt[:], in_=x_d[st * 128:(st + 1) * 128, :])
                        tp = psA.tile([128, D], F32, tag="tp")  # 8 transposes of this seq tile
                        for d in range(DCH):
                            nc.tensor.matmul(
                                r(tp[:, d * 128:(d + 1) * 128]),
                                r(xt[:, d * 128:(d + 1) * 128]),
                                r(ident[:]),
                                is_transpose=True,
                                start=True, stop=True,
                            )
                        # evict to xT: out element (p, d, c) -> xT[p, d*S + st*128 + c]
                        dst = xT3[:, :, st * 128: st * 128 + 128]
                        src = tp[:].rearrange("p (d c) -> p d c", d=DCH, c=128)
                        if st % 2 == 0:
                            nc.scalar.copy(out=dst, in_=src)
                        else:
                            nc.vector.tensor_copy(dst, src)

                # projections: qT, kT  (out [dims, seq])
                with tc.tile_pool(name="psB", bufs=3, space="PSUM") as psB:
                    for name, w_d, dstT in (("q", wq_d, qT), ("k", wk_d, kT)):
                        w_sb = [wp.tile([128, 256], F32, tag="w", name="w") for _ in range(DCH)]
                        for d in range(DCH):
                            nc.sync.dma_start(out=w_sb[d][:], in_=w_d[d * 128:(d + 1) * 128, :])
                        for pt in range(2):
                            for sc in range(NIC):
                                ps = psB.tile([128, 512], F32, tag="proj")
                                for d in range(DCH):
                                    nc.tensor.matmul(
                                        ps[:],
                                        r(w_sb[d][:, pt * 128:(pt + 1) * 128]),
                                        r(xT[:, d * S + sc * 512: d * S + sc * 512 + 512]),
                                        start=(d == 0), stop=(d == DCH - 1),
                                    )
                                dst = dstT[pt][:, sc * 512:(sc + 1) * 512]
                                if sc % 2 == 0:
                                    nc.scalar.copy(out=dst, in_=ps[:])
                                else:
                                    nc.vector.tensor_copy(dst, ps[:])

                    # projection: v (natural [seq, dims]), strided into vt + ones col
                    w_sb = [wp.tile([128, 256], F32, tag="w", name="w") for _ in range(DCH)]
                    for d in range(DCH):
                        nc.sync.dma_start(out=w_sb[d][:], in_=wv_d[d * 128:(d + 1) * 128, :])
                    for st in range(SEQT):
                        ps = psB.tile([128, 256], F32, tag="vproj")
                        for d in range(DCH):
                            nc.tensor.matmul(
                                ps[:],
                                r(xT[:, d * S + st * 128: d * S + st * 128 + 128]),
                                r(w_sb[d][:]),
                                start=(d == 0), stop=(d == DCH - 1),
                            )
                        nc.gpsimd.memset(vt[st][:], 1.0)  # ones cols preset
                        dst = vt[st][:].rearrange("p (h c) -> p h c", h=NHC, c=65)[:, :, :64]
                        src = ps[:].rearrange("p (h c) -> p h c", h=NHC, c=64)
                        nc.vector.tensor_copy(dst, src)

            # ---------------- Phase B2: rotary on qT/kT --------------------
            with (
                tc.tile_pool(name="cs", bufs=1) as csp,
                tc.tile_pool(name="rtmp", bufs=2) as rp,
            ):
                ct_sb = csp.tile([128, S], F32, tag="ct")
                st_sb = csp.tile([128, S], F32, tag="st")
                nc.sync.dma_start(out=ct_sb[:], in_=ct_d[:])
                nc.sync.dma_start(out=st_sb[:], in_=st_d[:])
                for t in (qT[0], qT[1], kT[0], kT[1]):
                    sw = rp.tile([128, S], F32, tag="sw")
                    nc.vector.stream_shuffle(sw[:], t[:], SWAP_MASK)
                    nc.vector.tensor_mul(t[:], t[:], ct_sb[:])
                    nc.vector.tensor_mul(sw[:], sw[:], st_sb[:])
                    nc.gpsimd.tensor_add(t[:], t[:], sw[:])

            # ---------------- Phase C+D: attention + out projection --------
            with (
                tc.tile_pool(name="epool", bufs=20) as ep,
                tc.tile_pool(name="opool", bufs=3) as op,
                tc.tile_pool(name="psL", bufs=3, space="PSUM") as psL,
                tc.tile_pool(name="psY", bufs=2, space="PSUM") as psY,
                tc.tile_pool(name="psO", bufs=2, space="PSUM") as psO,
            ):
                for ic in range(NIC):
                    i0 = ic * 512
                    for h in range(NHC):
                        pt, hh = h // 2, h % 2
                        r0 = hh * 64
                        njt = 4 * ic + 4
                        yt_ps = psY.tile([65, 512], F32, tag="yt")
                        for jt in range(njt):
                            l_ps = psL.tile([128, 512], F32, tag="l")
                            nc.tensor.matmul(
                                l_ps[:],
                                r(kT[pt][r0:r0 + 64, jt * 128:(jt + 1) * 128]),
                                r(qT[pt][r0:r0 + 64, i0:i0 + 512]),
                                start=True, stop=True,
                            )
                            e = ep.tile([128, 512], F32, tag="e")
                            nc.scalar.activation(e[:], l_ps[:], AF.Exp, scale=0.125)
                            if jt >= 4 * ic:  # diagonal-crossing tile: zero j > i
                                nc.gpsimd.affine_select(
                                    out=e[:], in_=e[:],
                                    compare_op=ALU.is_ge, fill=0.0,
                                    base=i0 - jt * 128,
                                    channel_multiplier=-1,
                                    pattern=[[1, 512]],
                                )
                            nc.tensor.matmul(
                                yt_ps[:],
                                r(vt[jt][:, h * 65: h * 65 + 65]),
                                r(e[:]),
                                start=(jt == 0), stop=(jt == njt - 1),
                            )
                        rs = sp.tile([1, 512], F32, tag="rs")
                        nc.vector.reciprocal(rs[0:1, :], yt_ps[64:65, :])
                        bc = sp.tile([64, 512], F32, tag="bc")
                        nc.gpsimd.partition_broadcast(bc[:], rs[0:1, :])
                        nc.vector.tensor_mul(
                            yT[pt][r0:r0 + 64, i0:i0 + 512], yt_ps[0:64, :], bc[:]
                        )
                    # out projection for the 4 seq tiles of this i-chunk
                    for st in range(4 * ic, 4 * ic + 4):
                        for dc in range(2):
                            ps = psO.tile([128, 512], F32, tag="o")
                            for pt in range(2):
                                nc.tensor.matmul(
                                    ps[:],
                                    r(yT[pt][:, st * 128:(st + 1) * 128]),
                                    r(wo_sb[pt][:, dc * 512:(dc + 1) * 512]),
                                    start=(pt == 0), stop=(pt == 1),
                                )
                            ob = op.tile([128, 512], F32, tag="ob")
                            if (st + dc) % 2 == 0:
                                nc.scalar.copy(out=ob[:], in_=ps[:])
                            else:
                                nc.vector.tensor_copy(ob[:], ps[:])
                            nc.sync.dma_start(
                                out=out_d[st * 128:(st + 1) * 128, dc * 512:(dc + 1) * 512],
                                in_=ob[:],
                            )

    nc.compile()
    return nc


def _in_maps(x, Wq, Wk, Wv, Wo):
    maps = []
    for core in range(8):
        b, hg = core // 4, core % 4
        c0 = hg * 4 * HD
        maps.append({
            "x": np.ascontiguousarray(x[b]),
            "wq": np.ascontiguousarray(Wq[:, c0:c0 + 256]),
            "wk": np.ascontiguousarray(Wk[:, c0:c0 + 256]),
            "wv": np.ascontiguousarray(Wv[:, c0:c0 + 256]),
            "wo": np.ascontiguousarray(Wo[c0:c0 + 256, :]),
        })
    return maps


def kernel(x, mask, Wq, Wk, Wv, Wo):
    from concourse.bass_utils import run_bass_kernel_spmd

    x, Wq, Wk, Wv, Wo = (np.asarray(a, np.float32) for a in (x, Wq, Wk, Wv, Wo))
    if "nc" not in _CACHED:
        _CACHED["nc"] = build_nc()
    res = run_bass_kernel_spmd(_CACHED["nc"], _in_maps(x, Wq, Wk, Wv, Wo),
                               core_ids=list(range(8)))
    out = np.zeros((2, S, D), np.float32)
    for core in range(8):
        out[core // 4] += res.results[core]["out"]
    return out


# revision 21
# speedup vs baseline: 14303.8025x; 14303.8025x over previous
"""Bass/Trainium2 kernel for a 16-head causal MHA block with partial rotary.

Problem shapes (hardcoded): x [2,2048,1024] fp32, Wq/Wk/Wv/Wo [1024,1024],
mask = causal tril [2048,2048] (hardcoded causality; mask input unused).

Sharding over 8 NeuronCores: core c handles batch c//4 and the 4 heads
h0 = (c%4)*4 .. h0+3 (tensor parallel on heads).  Each core computes its
partial output y_h @ Wo[h-block] summed over its 4 heads; the host adds the
4 per-batch partials.

Device-side plan (per core):
  A)  x [2048,1024] -> xT in SBUF via PE transposes (f32r transpose mode).
  B)  qT/kT = (Wslice^T @ xT) in [dims, seq] layout, v in [seq, dims] layout
      (+1.0 ones column per head for fused softmax denominators).
  B2) rotary applied to qT/kT in transposed layout:
      rot = q*C + pairswap(q)*S with host-precomputed C/S tables
      (identity rows for the non-rotary half of each head).
  C)  per (head, 512-wide i-chunk): logits^T tiles [j=128, i=512] = k^T q
      (f32r matmuls), exp via ScalarE (scale=1/8 folded in, no max
      subtraction -- logits are O(1)), causal zeroing of diagonal tiles via
      gpsimd affine_select, AV matmul with the ones column producing
      [65, 512] = [y^T ; colsums], then normalize by broadcasting 1/s.
  D)  output projection out = y^T.T @ Wo accumulated over the 4 local heads
      in PSUM, evicted and DMA'd to DRAM.
"""

import numpy as np

S, D, H, HD, PROT = 2048, 1024, 16, 64, 32
NHC = 4            # heads per core
SEQT = S // 128    # 16
DCH = D // 128     # 8
NIC = 4            # i-chunks of 512

_CACHED = {}


def _rot_tables():
    invf = 10000.0 ** (-np.arange(0, PROT, 2, dtype=np.float64) / PROT)  # [16]
    ang = np.arange(S, dtype=np.float64)[None, :] * invf[:, None]        # [16, S]
    C64 = np.ones((64, S), np.float64)
    S64 = np.zeros((64, S), np.float64)
    for d in range(PROT):
        C64[d] = np.cos(ang[d // 2])
        S64[d] = (1.0 if d % 2 else -1.0) * np.sin(ang[d // 2])
    CT = np.concatenate([C64, C64], 0).astype(np.float32)
    ST = np.concatenate([S64, S64], 0).astype(np.float32)
    return CT, ST


def build_nc(reps=1, ablate=(), psb=5, vpb=2, rotb=4, psl=2, epb=20, psy=2, pso=2):
    import concourse.bacc as bacc
    import concourse.mybir as mybir
    from concourse.tile import TileContext

    F32 = mybir.dt.float32
    F32R = mybir.dt.float32r
    AF = mybir.ActivationFunctionType
    ALU = mybir.AluOpType

    # MMDT: dtype of all matmul-feeding SBUF tensors. float32r streams at full
    # PE rate (1 cyc/row vs 4 for fp32) with ~1.5e-4 matmul rel err.
    MM_MODE = "f32r"
    MMDT = F32R if MM_MODE == "f32r" else F32

    def r(ap):  # transposes stay plain fp32 (exact); helper kept as identity
        return ap

    nc = bacc.Bacc("TRN2", target_bir_lowering=False, debug=False)

    x_d = nc.dram_tensor("x", [S, D], MMDT, kind="ExternalInput").ap()
    wq_d = nc.dram_tensor("wq", [D, 256], MMDT, kind="ExternalInput").ap()
    wk_d = nc.dram_tensor("wk", [D, 256], MMDT, kind="ExternalInput").ap()
    wv_d = nc.dram_tensor("wv", [D, 256], MMDT, kind="ExternalInput").ap()
    wo_d = nc.dram_tensor("wo", [256, D], MMDT, kind="ExternalInput").ap()
    out_d = nc.dram_tensor("out", [S, D], F32, kind="ExternalOutput").ap()

    CT, ST = _rot_tables()
    ct_d = nc.inline_tensor(CT, "ct_const").ap()
    st_d = nc.inline_tensor(ST, "st_const").ap()
    id_d = nc.inline_tensor(np.eye(128, dtype=np.float32), "id_const").ap()
    ones_d = nc.inline_tensor(np.ones((128, NHC), np.float32), "ones_const").ap()

    SWAP_MASK = [i ^ 1 for i in range(32)]

    with TileContext(nc) as tc:
      for _rep in range(reps):
        with (
            tc.tile_pool(name="persist", bufs=1) as pp,
            tc.tile_pool(name="small", bufs=2) as sp,
        ):
            qT = [pp.tile([128, S], MMDT, tag=f"qT{i}", name=f"qT{i}") for i in range(2)]
            kT = [pp.tile([128, S], MMDT, tag=f"kT{i}", name=f"kT{i}") for i in range(2)]
            vt = [pp.tile([128, NHC * 65], MMDT, tag=f"vt{i}", name=f"vt{i}") for i in range(SEQT)]
            wo_sb = [pp.tile([128, D], MMDT, tag=f"wo{i}", name=f"wo{i}") for i in range(2)]
            yT = [pp.tile([128, S], MMDT, tag=f"yT{i}", name=f"yT{i}") for i in range(2)]
            ident = pp.tile([128, 128], MMDT, tag="ident")
            nc.sync.dma_start(out=ident[:], in_=id_d[:].bitcast(MMDT))
            ones_sb = pp.tile([128, NHC], MMDT, tag="ones_sb")
            nc.sync.dma_start(out=ones_sb[:], in_=ones_d[:].bitcast(MMDT))
            for i in range(2):
                nc.sync.dma_start(out=wo_sb[i][:], in_=wo_d[i * 128:(i + 1) * 128, :])

            # ---------------- Phase A+B: transpose x, projections ----------
            with (
                tc.tile_pool(name="xnat", bufs=2) as xp,
                tc.tile_pool(name="xT", bufs=1) as xTp,
                tc.tile_pool(name="wts", bufs=8) as wp,
            ):
                xT = xTp.tile([128, DCH * S], MMDT, tag="xT")  # chunk d at cols [d*S, (d+1)*S)
                xT3 = xT[:].rearrange("p (d s) -> p d s", d=DCH, s=S)

                with tc.tile_pool(name="psA", bufs=3, space="PSUM") as psA:
                    for st in range(SEQT):
                        xt = xp.tile([128, D], MMDT, tag="x")
                        nc.sync.dma_start(out=xt[:], in_=x_d[st * 128:(st + 1) * 128, :])
                        tp = psA.tile([128, D], MMDT, tag="tp")  # 8 transposes of this seq tile
                        for d in range(DCH):
                            nc.tensor.matmul(
                                r(tp[:, d * 128:(d + 1) * 128]),
                                r(xt[:, d * 128:(d + 1) * 128]),
                                r(ident[:]),
                                is_transpose=True,
                                start=True, stop=True,
                            )
                        # evict to xT: out element (p, d, c) -> xT[p, d*S + st*128 + c]
                        dst = xT3[:, :, st * 128: st * 128 + 128]
                        src = tp[:].rearrange("p (d c) -> p d c", d=DCH, c=128)
                        if st % 2 == 0:
                            nc.scalar.copy(out=dst, in_=src)
                        else:
                            nc.vector.tensor_copy(dst, src)

                # C/S rotary tables loaded up front (used during q/k eviction)
                ct_sb = wp.tile([128, S], F32, tag="ct", name="ct", bufs=1)
                st_sb = wp.tile([128, S], F32, tag="st", name="st", bufs=1)
                nc.sync.dma_start(out=ct_sb[:], in_=ct_d[:])
                nc.sync.dma_start(out=st_sb[:], in_=st_d[:])

                with (
                    tc.tile_pool(name="psB", bufs=psb, space="PSUM") as psB,
                    tc.tile_pool(name="rot", bufs=rotb) as rp,
                ):
                    # projection: v first (natural [seq, dims]) so attention
                    # ic=0 can start as early as possible
                    w_sb = [wp.tile([128, 256], MMDT, tag="w", name="w") for _ in range(DCH)]
                    for d in range(DCH):
                        nc.sync.dma_start(out=w_sb[d][:], in_=wv_d[d * 128:(d + 1) * 128, :])
                    for st in range(SEQT):
                        ps = psB.tile([128, 256], F32, tag="vproj", bufs=vpb)
                        for d in range(DCH):
                            nc.tensor.matmul(
                                ps[:],
                                r(xT[:, d * S + st * 128: d * S + st * 128 + 128]),
                                r(w_sb[d][:]),
                                start=(d == 0), stop=(d == DCH - 1),
                            )
                        vt_ones = vt[st][:].rearrange("p (h c) -> p h c", h=NHC, c=65)[:, :, 64:65]
                        nc.vector.tensor_copy(vt_ones, ones_sb[:].rearrange("p (h c) -> p h c", h=NHC, c=1))
                        dst = vt[st][:].rearrange("p (h c) -> p h c", h=NHC, c=65)[:, :, :64]
                        src = ps[:].rearrange("p (h c) -> p h c", h=NHC, c=64)
                        nc.vector.tensor_copy(dst, src)

                    # projections qT/kT with rotary fused into the eviction:
                    #   t0 = psum (ACT copy), sw = pairswap(t0) (DVE shuffle)
                    #   t0 *= C (DVE), sw *= S (POOL), qT = t0 + sw (POOL, f32r)
                    for name, w_d, dstT in (("q", wq_d, qT), ("k", wk_d, kT)):
                        w_sb = [wp.tile([128, 256], MMDT, tag="w", name="w") for _ in range(DCH)]
                        for d in range(DCH):
                            nc.sync.dma_start(out=w_sb[d][:], in_=w_d[d * 128:(d + 1) * 128, :])
                        for pt in range(2):
                            for sc in range(NIC):
                                ps = psB.tile([128, 512], F32, tag="proj")
                                for d in range(DCH):
                                    nc.tensor.matmul(
                                        ps[:],
                                        r(w_sb[d][:, pt * 128:(pt + 1) * 128]),
                                        r(xT[:, d * S + sc * 512: d * S + sc * 512 + 512]),
                                        start=(d == 0), stop=(d == DCH - 1),
                                    )
                                dst = dstT[pt][:, sc * 512:(sc + 1) * 512]
                                if "rotary" in ablate:
                                    nc.scalar.copy(out=dst, in_=ps[:])
                                else:
                                    t0 = rp.tile([128, 512], F32, tag="t0", name="t0")
                                    sw = rp.tile([128, 512], F32, tag="sw", name="sw")
                                    nc.scalar.copy(out=t0[:], in_=ps[:])
                                    nc.vector.stream_shuffle(sw[:], t0[:], SWAP_MASK)
                                    nc.vector.tensor_mul(
                                        t0[:], t0[:], ct_sb[:, sc * 512:(sc + 1) * 512])
                                    nc.gpsimd.tensor_mul(
                                        sw[:], sw[:], st_sb[:, sc * 512:(sc + 1) * 512])
                                    nc.gpsimd.tensor_add(dst, t0[:], sw[:])

            # ---------------- Phase C+D: attention + out projection --------
            with (
                tc.tile_pool(name="epool", bufs=epb) as ep,
                tc.tile_pool(name="opool", bufs=3) as op,
                tc.tile_pool(name="psL", bufs=psl, space="PSUM") as psL,
                tc.tile_pool(name="psY", bufs=psy, space="PSUM") as psY,
                tc.tile_pool(name="psO", bufs=pso, space="PSUM") as psO,
            ):
                def emit_qk_block(ic, h):
                    """QK matmuls + exp (+causal mask) for one (i-chunk, head).
                    Returns state consumed later by emit_av_block."""
                    i0 = ic * 512
                    njt = 4 * ic + 4
                    pt, hh = h // 2, h % 2
                    r0 = hh * 64
                    yt_ps = psY.tile([65, 512], F32, tag="yt", name="yt")
                    es = []
                    for jp in range(njt // 2):   # pairs of j-tiles
                        l_ps = psL.tile([128, 1024], F32, tag="l", name="l")
                        e = ep.tile([128, 1024], MMDT, tag="e", name="e")
                        for u in range(2):
                            jt = 2 * jp + u
                            if "qk" not in ablate:
                                nc.tensor.matmul(
                                    l_ps[:, u * 512:(u + 1) * 512],
                                    r(kT[pt][r0:r0 + 64, jt * 128:(jt + 1) * 128]),
                                    r(qT[pt][r0:r0 + 64, i0:i0 + 512]),
                                    start=True, stop=True,
                                )
                        if "exp" in ablate:
                            nc.vector.tensor_copy(e[:], l_ps[:])
                        else:
                            nc.scalar.activation(e[:], l_ps[:], AF.Exp, scale=0.125)
                        for u in range(2):
                            jt = 2 * jp + u
                            if jt >= 4 * ic and "affine" not in ablate:
                                # only [0, w+128) of the tile can be masked
                                w = jt * 128 - i0
                                nc.gpsimd.affine_select(
                                    out=e[:, u * 512: u * 512 + w + 128],
                                    in_=e[:, u * 512: u * 512 + w + 128],
                                    compare_op=ALU.is_ge, fill=0.0,
                                    base=-w, channel_multiplier=-1,
                                    pattern=[[1, w + 128]],
                                )
                        es.append(e)
                    return (ic, h, yt_ps, es)

                def emit_av_block(state):
                    """AV accumulation + normalization for a block emitted by
                    emit_qk_block; software-pipelined one block behind so the
                    PE streams this block's AVs while ACT runs the next
                    block's exps."""
                    ic, h, yt_ps, es = state
                    i0 = ic * 512
                    njt = 4 * ic + 4
                    pt, hh = h // 2, h % 2
                    r0 = hh * 64
                    for jp, e in enumerate(es):
                        for u in range(2):
                            jt = 2 * jp + u
                            if "av" not in ablate:
                                nc.tensor.matmul(
                                    yt_ps[:],
                                    r(vt[jt][:, h * 65: h * 65 + 65]),
                                    r(e[:, u * 512:(u + 1) * 512]),
                                    start=(jt == 0), stop=(jt == njt - 1),
                                )
                    if "norm" not in ablate:
                        rs = sp.tile([1, 512], F32, tag="rs", name="rs")
                        nc.vector.reciprocal(rs[0:1, :], yt_ps[64:65, :])
                        bc = sp.tile([64, 512], F32, tag="bc", name="bc")
                        nc.gpsimd.partition_broadcast(bc[:], rs[0:1, :])
                        nc.vector.tensor_mul(
                            yT[pt][r0:r0 + 64, i0:i0 + 512], yt_ps[0:64, :], bc[:]
                        )
                    if h == NHC - 1:
                        emit_oproj(ic)

                def emit_oproj(ic):
                    for st in range(4 * ic, 4 * ic + 4):
                        for dc in range(2):
                            ps = psO.tile([128, 512], F32, tag="o", name="o")
                            for pt in range(2):
                                nc.tensor.matmul(
                                    ps[:],
                                    r(yT[pt][:, st * 128:(st + 1) * 128]),
                                    r(wo_sb[pt][:, dc * 512:(dc + 1) * 512]),
                                    start=(pt == 0), stop=(pt == 1),
                                )
                            ob = op.tile([128, 512], F32, tag="ob", name="ob")
                            nc.vector.tensor_copy(ob[:], ps[:])
                            nc.sync.dma_start(
                                out=out_d[st * 128:(st + 1) * 128, dc * 512:(dc + 1) * 512],
                                in_=ob[:],
                            )

                pending = None
                for ic in range(NIC):
                    for h in range(NHC):
                        state = emit_qk_block(ic, h)
                        if pending is not None:
                            emit_av_block(pending)
                        pending = state
                emit_av_block(pending)

    nc.compile()
    return nc


def _in_maps(x, Wq, Wk, Wv, Wo):
    maps = []
    for core in range(8):
        b, hg = core // 4, core % 4
        c0 = hg * 4 * HD
        maps.append({
            "x": np.ascontiguousarray(x[b]),
            "wq": np.ascontiguousarray(Wq[:, c0:c0 + 256]),
            "wk": np.ascontiguousarray(Wk[:, c0:c0 + 256]),
            "wv": np.ascontiguousarray(Wv[:, c0:c0 + 256]),
            "wo": np.ascontiguousarray(Wo[c0:c0 + 256, :]),
        })
    return maps


def kernel(x, mask, Wq, Wk, Wv, Wo):
    from concourse.bass_utils import run_bass_kernel_spmd

    x, Wq, Wk, Wv, Wo = (np.asarray(a, np.float32) for a in (x, Wq, Wk, Wv, Wo))
    if "nc" not in _CACHED:
        _CACHED["nc"] = build_nc()
    res = run_bass_kernel_spmd(_CACHED["nc"], _in_maps(x, Wq, Wk, Wv, Wo),
                               core_ids=list(range(8)))
    out = np.zeros((2, S, D), np.float32)
    for core in range(8):
        out[core // 4] += res.results[core]["out"]
    return out
